# revision 4
# baseline (speedup 1.0000x reference)
"""Bass/Tile Trainium2 kernel for BinaryMultiHeadAttention (B=2, S=2048, D=1024, H=16).

Sharding: token-parallel across 8 cores. Core c handles batch c//4, tokens
(c%4)*512..+512 of that batch. Q/K/V projections are computed for the core's
own 512 tokens only; K (feature-major) and V (token-major) are AllGathered
within each 4-core batch group so every core sees its batch's full 2048
tokens for attention. The squared-softmax p^2/sum(p^2) is computed exactly as
softmax(2*scores) = exp(dot/4)/sum(exp(dot/4)) (dot is an integer in [0,64],
so no max subtraction is needed). The ones-column trick makes one PV matmul
produce both the attention numerator and the softmax denominator.

Self-contained: hardcodes shapes; builds + compiles the Bass program once per
process and runs it SPMD on cores 0-7.
"""

import numpy as np
import ml_dtypes

B, S, D, H, HD = 2, 2048, 1024, 16, 64
TPC = 512  # tokens per core
NCORES = 8
GROUPS = [[0, 1, 2, 3], [4, 5, 6, 7]]

_CACHE = {}


def _build_program():
    import concourse.mybir as mybir
    import concourse.tile as tile
    from concourse import bacc

    F32 = mybir.dt.float32
    BF16 = mybir.dt.bfloat16
    AF = mybir.ActivationFunctionType
    GT = mybir.AluOpType.is_gt
    MULT = mybir.AluOpType.mult

    nc = bacc.Bacc("TRN2", target_bir_lowering=False, debug=False, num_devices=NCORES)

    xT = nc.dram_tensor("xT", [D, TPC], BF16, kind="ExternalInput")
    wqT = nc.dram_tensor("wqT", [D, D], BF16, kind="ExternalInput")
    wkT = nc.dram_tensor("wkT", [D, D], BF16, kind="ExternalInput")
    wvT = nc.dram_tensor("wvT", [D, D], BF16, kind="ExternalInput")
    woT = nc.dram_tensor("woT", [D, D], BF16, kind="ExternalInput")
    thrq = nc.dram_tensor("thrq", [128, 8], F32, kind="ExternalInput")
    thrk = nc.dram_tensor("thrk", [128, 8], F32, kind="ExternalInput")
    thrvb = nc.dram_tensor("thrvb", [128, D], F32, kind="ExternalInput")
    throb = nc.dram_tensor("throb", [128, D], F32, kind="ExternalInput")
    y = nc.dram_tensor("y", [TPC, D], F32, kind="ExternalOutput")

    with tile.TileContext(nc) as tc:
        with (
            tc.tile_pool(name="w", bufs=2) as wpool,
            tc.tile_pool(name="big", bufs=1) as bigpool,
            tc.tile_pool(name="consts", bufs=1) as cpool,
            tc.tile_pool(name="stage", bufs=4) as stpool,
            tc.tile_pool(name="kt", bufs=2) as ktpool,
            tc.tile_pool(name="vh", bufs=2) as vhpool,
            tc.tile_pool(name="p", bufs=2) as ppool,
            tc.tile_pool(name="nrm", bufs=4) as nrmpool,
            tc.tile_pool(name="bp", bufs=2) as bppool,
            tc.tile_pool(name="yo", bufs=3) as yopool,
            tc.tile_pool(name="dram", bufs=1, space="DRAM") as drpool,
            tc.tile_pool(name="ps_s", bufs=1, space="PSUM") as ps_s,
            tc.tile_pool(name="ps_pv", bufs=2, space="PSUM") as ps_pv,
            tc.tile_pool(name="ps_mm", bufs=2, space="PSUM") as ps_mm,
        ):
            # ---- constants
            thrq_sb = cpool.tile([128, 8], F32, tag="thrq")
            nc.sync.dma_start(thrq_sb[:], thrq[:, :])
            thrk_sb = cpool.tile([128, 8], F32, tag="thrk")
            nc.sync.dma_start(thrk_sb[:], thrk[:, :])
            thrvb_sb = cpool.tile([128, D], F32, tag="thrvb")
            nc.sync.dma_start(thrvb_sb[:], thrvb[:, :])
            throb_sb = cpool.tile([128, D], F32, tag="throb")
            nc.sync.dma_start(throb_sb[:], throb[:, :])
            # ones rows at partition bases 0 and 32 (for the K=1 broadcast
            # matmuls; partition bases must be in {0, 32, 64, 96})
            ones_sb = cpool.tile([33, 64], F32, tag="ones")
            nc.vector.memset(ones_sb[:], 1.0)

            # ---- load x and K/V weights
            xt = bigpool.tile([128, 8, TPC], BF16, tag="xt")
            nc.sync.dma_start(xt[:], xT[:, :].rearrange("(c p) t -> p c t", p=128))
            wk_sb = wpool.tile([128, 8, D], BF16, tag="w")
            nc.sync.dma_start(wk_sb[:], wkT[:, :].rearrange("(c p) f -> p c f", p=128))
            wv_sb = wpool.tile([128, 8, D], BF16, tag="w")
            nc.sync.dma_start(wv_sb[:], wvT[:, :].rearrange("(c p) f -> p c f", p=128))

            ag_k_in = drpool.tile([D, TPC], BF16, tag="agki")
            ag_v_in = drpool.tile([TPC, D], BF16, tag="agvi")
            ag_k_out = drpool.tile([4 * D, TPC], BF16, tag="agko")
            ag_v_out = drpool.tile([4 * TPC, D], BF16, tag="agvo")

            # ---- K projection: KT_c [1024 f, 512 t] binary, feature-major
            for jf in range(8):
                ps = ps_mm.tile([128, 512], F32, tag="mm")
                for dc in range(8):
                    nc.tensor.matmul(
                        ps[:],
                        lhsT=wk_sb[:, dc, jf * 128 : (jf + 1) * 128],
                        rhs=xt[:, dc, :],
                        start=(dc == 0),
                        stop=(dc == 7),
                    )
                st = stpool.tile([128, TPC], BF16, tag="st")
                nc.vector.tensor_scalar(
                    out=st[:],
                    in0=ps[:],
                    scalar1=thrk_sb[:, jf : jf + 1],
                    scalar2=None,
                    op0=GT,
                )
                nc.sync.dma_start(ag_k_in[jf * 128 : (jf + 1) * 128, :], st[:])

            # ---- V projection: V_c [512 t, 1024 f] binary, token-major
            for tt in range(4):
                for fh in range(2):
                    ps = ps_mm.tile([128, 512], F32, tag="mm")
                    for dc in range(8):
                        nc.tensor.matmul(
                            ps[:],
                            lhsT=xt[:, dc, tt * 128 : (tt + 1) * 128],
                            rhs=wv_sb[:, dc, fh * 512 : (fh + 1) * 512],
                            start=(dc == 0),
                            stop=(dc == 7),
                        )
                    st = stpool.tile([128, TPC], BF16, tag="st")
                    nc.vector.tensor_tensor(
                        out=st[:],
                        in0=ps[:],
                        in1=thrvb_sb[:, fh * 512 : (fh + 1) * 512],
                        op=GT,
                    )
                    nc.sync.dma_start(
                        ag_v_in[tt * 128 : (tt + 1) * 128, fh * 512 : (fh + 1) * 512],
                        st[:],
                    )

            # ---- AllGather K and V within each batch group of 4 cores
            nc.gpsimd.collective_compute(
                "AllGather",
                mybir.AluOpType.bypass,
                replica_groups=GROUPS,
                ins=[ag_k_in.opt()],
                outs=[ag_k_out.opt()],
            )
            nc.gpsimd.collective_compute(
                "AllGather",
                mybir.AluOpType.bypass,
                replica_groups=GROUPS,
                ins=[ag_v_in.opt()],
                outs=[ag_v_out.opt()],
            )

            # ---- Q projection (overlaps the collectives)
            wq_sb = wpool.tile([128, 8, D], BF16, tag="w")
            nc.sync.dma_start(wq_sb[:], wqT[:, :].rearrange("(c p) f -> p c f", p=128))
            qt = bigpool.tile([128, 8, TPC], BF16, tag="qt")
            for jf in range(8):
                ps = ps_mm.tile([128, 512], F32, tag="mm")
                for dc in range(8):
                    nc.tensor.matmul(
                        ps[:],
                        lhsT=wq_sb[:, dc, jf * 128 : (jf + 1) * 128],
                        rhs=xt[:, dc, :],
                        start=(dc == 0),
                        stop=(dc == 7),
                    )
                nc.vector.tensor_scalar(
                    out=qt[:, jf, :],
                    in0=ps[:],
                    scalar1=thrq_sb[:, jf : jf + 1],
                    scalar2=None,
                    op0=GT,
                )

            wo_sb = wpool.tile([128, 8, D], BF16, tag="w")
            nc.sync.dma_start(wo_sb[:], woT[:, :].rearrange("(c p) f -> p c f", p=128))

            at = bigpool.tile([128, 8, TPC], BF16, tag="at")
            kt_view = ag_k_out[:, :].rearrange("(r f) t -> f r t", r=4)
            vh_view = ag_v_out[:, :].rearrange("(kc p) f -> p kc f", p=128)

            # ---- attention, head pair jj = heads (2jj, 2jj+1)
            for jj in range(8):
                kt = ktpool.tile([128, 4, TPC], BF16, tag="kt")
                nc.sync.dma_start(kt[:], kt_view[jj * 128 : (jj + 1) * 128, :, :])
                pv_tiles = []
                den = nrmpool.tile([33, 512], F32, tag="den")
                for hp in range(2):
                    h = 2 * jj + hp
                    vh = vhpool.tile([128, 16, 65], BF16, tag="vh")
                    nc.gpsimd.memset(vh[:], 1.0)
                    nc.sync.dma_start(
                        vh[:, :, 0:64], vh_view[:, :, h * 64 : (h + 1) * 64]
                    )
                    p_t = ppool.tile([128, 16 * 512], BF16, tag="p")
                    for g in range(4):
                        sc = ps_s.tile([128, 2048], F32, tag="sc")
                        for s2 in range(4):
                            kcc = 4 * g + s2
                            r, lc = kcc // 4, kcc % 4
                            nc.tensor.matmul(
                                sc[:, s2 * 512 : (s2 + 1) * 512],
                                lhsT=kt[
                                    hp * 64 : (hp + 1) * 64,
                                    r,
                                    lc * 128 : (lc + 1) * 128,
                                ],
                                rhs=qt[hp * 64 : (hp + 1) * 64, jj, :],
                                start=True,
                                stop=True,
                            )
                        # p = exp(dot/4); squared-renormalized softmax == softmax(2s)
                        nc.scalar.activation(
                            p_t[:, g * 2048 : (g + 1) * 2048],
                            sc[:],
                            AF.Exp,
                            bias=0.0,
                            scale=0.25,
                        )
                    pv = ps_pv.tile([65, 512], F32, tag="pv")
                    for kc in range(16):
                        nc.tensor.matmul(
                            pv[:],
                            lhsT=vh[:, kc, :],
                            rhs=p_t[:, kc * 512 : (kc + 1) * 512],
                            start=(kc == 0),
                            stop=(kc == 15),
                        )
                    nc.vector.tensor_copy(den[32 * hp : 32 * hp + 1, :], pv[64:65, :])
                    pv_tiles.append(pv)
                rec = nrmpool.tile([33, 512], F32, tag="rec")
                nc.vector.reciprocal(rec[:], den[:])
                for hp in range(2):
                    bpp = ps_mm.tile([128, 512], F32, tag="mm")
                    nc.tensor.matmul(
                        bpp[0:64, :],
                        lhsT=ones_sb[32 * hp : 32 * hp + 1, :],
                        rhs=rec[32 * hp : 32 * hp + 1, :],
                        start=True,
                        stop=True,
                    )
                    bps = bppool.tile([64, 512], F32, tag="bp")
                    nc.vector.tensor_copy(bps[:], bpp[0:64, :])
                    nc.vector.tensor_tensor(
                        out=at[hp * 64 : (hp + 1) * 64, jj, :],
                        in0=pv_tiles[hp][0:64, :],
                        in1=bps[:],
                        op=MULT,
                    )

            # ---- output projection + threshold
            for tt in range(4):
                for fh in range(2):
                    ps = ps_mm.tile([128, 512], F32, tag="mm")
                    for jj in range(8):
                        nc.tensor.matmul(
                            ps[:],
                            lhsT=at[:, jj, tt * 128 : (tt + 1) * 128],
                            rhs=wo_sb[:, jj, fh * 512 : (fh + 1) * 512],
                            start=(jj == 0),
                            stop=(jj == 7),
                        )
                    ys = yopool.tile([128, 512], F32, tag="y")
                    nc.vector.tensor_tensor(
                        out=ys[:],
                        in0=ps[:],
                        in1=throb_sb[:, fh * 512 : (fh + 1) * 512],
                        op=GT,
                    )
                    nc.sync.dma_start(
                        y[tt * 128 : (tt + 1) * 128, fh * 512 : (fh + 1) * 512], ys[:]
                    )

    nc.compile()
    return nc


def _get_program():
    if "nc" not in _CACHE:
        _CACHE["nc"] = _build_program()
    return _CACHE["nc"]


def _prep_inputs(x, wq, bq, wk, bk, wv, bv, wo, bo):
    bf16 = ml_dtypes.bfloat16
    x = np.asarray(x, dtype=np.float32)

    def binT(w):
        bw = np.clip(np.sign(np.asarray(w, dtype=np.float32)), 0.0, 1.0)
        return np.ascontiguousarray(bw.T).astype(bf16)

    shared = {
        "wqT": binT(wq),
        "wkT": binT(wk),
        "wvT": binT(wv),
        "woT": binT(wo),
        "thrq": np.ascontiguousarray(
            (0.5 - np.asarray(bq, np.float32)).reshape(8, 128).T
        ),
        "thrk": np.ascontiguousarray(
            (0.5 - np.asarray(bk, np.float32)).reshape(8, 128).T
        ),
        "thrvb": np.ascontiguousarray(
            np.tile((0.5 - np.asarray(bv, np.float32))[None, :], (128, 1))
        ),
        "throb": np.ascontiguousarray(
            np.tile((0.5 - np.asarray(bo, np.float32))[None, :], (128, 1))
        ),
    }
    in_maps = []
    for c in range(NCORES):
        b, blk = c // 4, c % 4
        xT_c = np.ascontiguousarray(x[b, blk * TPC : (blk + 1) * TPC, :].T).astype(bf16)
        m = dict(shared)
        m["xT"] = xT_c
        in_maps.append(m)
    return in_maps


def _gather_output(results):
    y = np.empty((B, S, D), dtype=np.float32)
    for c in range(NCORES):
        b, blk = c // 4, c % 4
        y[b, blk * TPC : (blk + 1) * TPC, :] = results[c]["y"]
    return y


def _run(in_maps, **kw):
    from concourse.bass_utils import run_bass_kernel_spmd

    nc = _get_program()
    return run_bass_kernel_spmd(nc, in_maps, list(range(NCORES)), **kw)


def kernel(x, wq, bq, wk, bk, wv, bv, wo, bo):
    in_maps = _prep_inputs(x, wq, bq, wk, bk, wv, bv, wo, bo)
    res = _run(in_maps)
    return _gather_output(res.results)


def run_traced(inputs, **kw):
    """For test.py: run with NTFF tracing, return (output, BassKernelResults)."""
    in_maps = _prep_inputs(**inputs)
    res = _run(in_maps, trace=True, **kw)
    return _gather_output(res.results), res
